# revision 15
# baseline (speedup 1.0000x reference)
"""Trainium2 Bass kernel for nn_Attention_local (sparse routed attention).

Math (per batch b, head h):
  qkv = x @ Wqkv ; q,k,v per head (d=64)
  top-49 routing indices per (b,h,query) from adj logits
  attention over the selected 49 keys; gelu; @ Wv

Device strategy (8 cores, data-parallel over batch, 2 batches/core):
  - Exact threshold mask instead of gather: theta = 49th-largest of
    adj[b,h,i,:]; keep = adj >= theta (exact: no boundary ties).
  - theta found by 3 damped quantile-model iterations (ACT Sign counting
    with per-partition bias + accumulate) + 1 exact DVE count, then an
    exact one-sided fixup: top-16-below-theta via max8/match_replace and
    a fused one-hot select. Bit-exact fp32 schedule validated on host.
  - s = (q*scale) @ k^T dense (bf16, PE), e = exp(s) on ACT (|s|<0.5),
    ep = (adj>=theta)*e fused with row-sum on DVE, normalize on GPSIMD
    (normalize_recip), attn transpose via DMA xbar, o^T on PE, batched
    gelu sweep on ACT, final projection on PE.
"""

import numpy as np
import ml_dtypes
from contextlib import ExitStack

import concourse.bass as bass
import concourse.tile as tile
from concourse import bacc, library_config, mybir
from concourse.bass_utils import run_bass_kernel_spmd

B, T, DIM = 16, 196, 512
H, D = 8, 64
TOPK = 49
NB = 2                 # batches per core
NPAIR = NB * H         # 16
NCORES = 8
TA = 128
TB = T - TA            # 68
NBF = 9                # flat-packed selection tiles for B rows (16*68 -> 9*128)
NBROWS = NPAIR * TB    # 1088
SCALE = DIM ** -0.5
BF = ml_dtypes.bfloat16
AF = mybir.ActivationFunctionType
ALU = mybir.AluOpType
NEG = -1.0e30

_SCHED = {}


def _sched():
    """Selection schedule constants (bit-exact fp32, validated on host)."""
    if _SCHED:
        return _SCHED
    from scipy.stats import norm
    cs = np.arange(15, 100)
    coef = np.polyfit(cs, norm.ppf(1 - cs / 196.0), 5).astype(np.float32)
    A5, A4, A3, A2, A1, A0 = [np.float32(a) for a in coef]

    def zf(x):
        c = np.float32(np.clip(x, 15.0, 99.0))
        r = A5
        for a in (A4, A3, A2, A1, A0):
            r = np.float32(r * c + a)
        return r

    TG = [49.0, 42.0, 41.0]
    DM = [1.0, 0.7, 0.5]
    Ks = [np.float32(np.float32(d) * zf(tg) - np.float32(d) * A0)
          for tg, d in zip(TG, DM)]
    _SCHED.update(dict(coef=(A5, A4, A3, A2, A1, A0), damps=DM, Ks=Ks,
                       theta0=np.float32(0.6744898)))
    return _SCHED


# selection column order: [A0..A7, Bf0..Bf4, A8..A15, Bf5..Bf8]
def _col_of_pair(p):
    return p if p < 8 else p + 5


def _col_of_bf(u):
    return 8 + u if u < 5 else 16 + u


_PROGRAM_CACHE = {}


def _build_program(gelu=True):
    f32, bf16 = mybir.dt.float32, mybir.dt.bfloat16
    sch = _sched()
    A5, A4, A3, A2, A1, A0 = sch["coef"]

    nc = bacc.Bacc("TRN2", target_bir_lowering=False, debug=False,
                   num_devices=NCORES)

    xT_d = nc.dram_tensor("xT", [4, 128, NB * T], bf16, kind="ExternalInput")
    wqk_d = nc.dram_tensor("wqk", [4, 128, 2 * DIM], bf16, kind="ExternalInput")
    wvp_d = nc.dram_tensor("wvp", [4, 128, DIM], bf16, kind="ExternalInput")
    wo_d = nc.dram_tensor("wo", [4, 128, DIM], bf16, kind="ExternalInput")
    adjA_d = nc.dram_tensor("adjA", [TA, NPAIR * T], f32, kind="ExternalInput")
    adjB_d = nc.dram_tensor("adjB", [TB, NPAIR * T], f32, kind="ExternalInput")
    adjBf_d = nc.dram_tensor("adjBf", [128, NBF * T], f32, kind="ExternalInput")
    io8_d = nc.dram_tensor("iota8", [128, 8], f32, kind="ExternalInput")
    out_d = nc.dram_tensor("out", [NB * T, DIM], f32, kind="ExternalOutput")

    with ExitStack() as ctx:
        tc = ctx.enter_context(tile.TileContext(nc))
        const = ctx.enter_context(tc.tile_pool(name="const", bufs=1))
        dram = ctx.enter_context(tc.tile_pool(name="dram", bufs=1, space="DRAM"))
        wk = ctx.enter_context(tc.tile_pool(name="wk", bufs=3))
        esb = ctx.enter_context(tc.tile_pool(name="esb", bufs=8))
        epsb = ctx.enter_context(tc.tile_pool(name="epsb", bufs=4))
        atsb = ctx.enter_context(tc.tile_pool(name="atsb", bufs=4))
        jsb = ctx.enter_context(tc.tile_pool(name="jsb", bufs=3))
        ps_mm = ctx.enter_context(tc.tile_pool(name="ps_mm", bufs=1, space="PSUM"))
        ps_s = ctx.enter_context(tc.tile_pool(name="ps_s", bufs=3, space="PSUM"))
        ps_o = ctx.enter_context(tc.tile_pool(name="ps_o", bufs=2, space="PSUM"))
        ps_f = ctx.enter_context(tc.tile_pool(name="ps_f", bufs=1, space="PSUM"))

        nc.gpsimd.load_library(library_config.attn)

        # ---------------- small loads first ----------------
        xT_sb = [const.tile([128, NB * T], bf16, name=f"xT{kc}", tag=f"xT{kc}") for kc in range(4)]
        wqk_sb = [const.tile([128, 2 * DIM], bf16, name=f"wqk{kc}", tag=f"wqk{kc}") for kc in range(4)]
        wvp_sb = [const.tile([128, DIM], bf16, name=f"wvp{kc}", tag=f"wvp{kc}") for kc in range(4)]
        wo_sb = [const.tile([128, DIM], bf16, name=f"wo{kc}", tag=f"wo{kc}") for kc in range(4)]
        iota8 = const.tile([128, 8], f32)
        nc.sync.dma_start(iota8[:], io8_d[:])
        for kc in range(4):
            nc.sync.dma_start(xT_sb[kc][:], xT_d[kc])
            nc.sync.dma_start(wqk_sb[kc][:], wqk_d[kc])
            nc.sync.dma_start(wvp_sb[kc][:], wvp_d[kc])
            nc.sync.dma_start(wo_sb[kc][:], wo_d[kc])

        adjA_sb = const.tile([TA, NPAIR * T], f32)
        adjB_sb = const.tile([TB, NPAIR * T], f32)
        adjBf_sb = const.tile([128, NBF * T], f32)

        # selection state
        th = const.tile([128, 25], f32)
        thneg = const.tile([128, 25], f32)
        sg = const.tile([128, 25], f32)
        cw = const.tile([128, 25], f32)
        rw = const.tile([128, 25], f32)
        rw2 = const.tile([128, 25], f32)
        tm1 = const.tile([128, 25], f32)
        tm9 = const.tile([128, 25], f32)
        tsa = const.tile([128, 25], f32)
        thstar = const.tile([128, 25], f32)
        mba = const.tile([128, 25 * 8], f32)
        mbb = const.tile([128, 25 * 8], f32)
        junkA = const.tile([128, T], f32)
        junkD = const.tile([128, T], f32)
        junk8 = const.tile([128, 8], f32)
        thbB = dram.tile([NBF * 128], f32)
        thB = const.tile([TB, NPAIR], f32)
        rsA = const.tile([TA, NPAIR], f32)
        rsB = const.tile([TB, NPAIR], f32)

        nc.gpsimd.memset(th[:], float(sch["theta0"]))
        nc.gpsimd.memset(thneg[:], -float(sch["theta0"]))

        # ---------------- q/k/v projections (PE) ----------------
        qkT_sb = [const.tile([D, NB * T], bf16, name=f"qkT{m}", tag=f"qkT{m}") for m in range(16)]
        for mt in range(8):
            ps = ps_mm.tile([128, NB * T], f32, name="qkps", tag="mm")
            for kc in range(4):
                nc.tensor.matmul(
                    ps[:], wqk_sb[kc][:, mt * 128:(mt + 1) * 128], xT_sb[kc][:],
                    start=(kc == 0), stop=(kc == 3))
            h0 = 2 * mt
            nc.scalar.activation(qkT_sb[h0][:], ps[0:D, :], AF.Copy)
            nc.scalar.activation(qkT_sb[h0 + 1][:], ps[D:128, :], AF.Copy)

        vA_sb = [const.tile([TA, DIM], bf16, name=f"vA{bi}", tag=f"vA{bi}") for bi in range(NB)]
        vB_sb = [const.tile([TB, DIM], bf16, name=f"vB{bi}", tag=f"vB{bi}") for bi in range(NB)]
        for bi in range(NB):
            psA = ps_mm.tile([TA, DIM], f32, name="vpsA", tag="mm")
            psB = ps_mm.tile([TB, DIM], f32, name="vpsB", tag="mm")
            for kc in range(4):
                c0 = bi * T
                nc.tensor.matmul(psA[:], xT_sb[kc][:, c0:c0 + TA], wvp_sb[kc][:],
                                 start=(kc == 0), stop=(kc == 3))
            for kc in range(4):
                c0 = bi * T + TA
                nc.tensor.matmul(psB[:], xT_sb[kc][:, c0:c0 + TB], wvp_sb[kc][:],
                                 start=(kc == 0), stop=(kc == 3))
            nc.scalar.activation(vA_sb[bi][:], psA[:], AF.Copy)
            nc.scalar.activation(vB_sb[bi][:], psB[:], AF.Copy)

        oT_sb = [const.tile([128, NB * T], f32, name=f"oT{kc}", tag=f"oT{kc}") for kc in range(4)]
        gT_sb = [const.tile([128, NB * T], bf16, name=f"gT{kc}", tag=f"gT{kc}") for kc in range(4)]

        def slice_ap(col):
            if col < 8:
                return adjA_sb[:, col * T:(col + 1) * T]
            if col < 13:
                u = col - 8
                return adjBf_sb[:, u * T:(u + 1) * T]
            if col < 21:
                p = col - 5
                return adjA_sb[:, p * T:(p + 1) * T]
            u = col - 16
            return adjBf_sb[:, u * T:(u + 1) * T]

        # ================= two stages of 8 pairs =================
        for st in range(2):
            c0s, c1s = (0, 13) if st == 0 else (13, 25)
            pairs = range(8 * st, 8 * st + 8)
            bfs = range(0, 5) if st == 0 else range(5, 9)
            cols = range(c0s, c1s)
            g = (slice(None), slice(c0s, c1s))

            # stage input DMAs
            for p in pairs:
                nc.sync.dma_start(adjA_sb[:, p * T:(p + 1) * T],
                                  adjA_d[:, p * T:(p + 1) * T])
            for u in bfs:
                nc.sync.dma_start(adjBf_sb[:, u * T:(u + 1) * T],
                                  adjBf_d[:, u * T:(u + 1) * T])
            for p in pairs:
                nc.sync.dma_start(adjB_sb[:, p * T:(p + 1) * T],
                                  adjB_d[:, p * T:(p + 1) * T])

            # ---- 3 damped quantile rounds (counting on ACT via Sign) ----
            for r in range(3):
                for col in cols:
                    nc.scalar.activation(
                        junkA[:], slice_ap(col), AF.Sign,
                        bias=thneg[:, col:col + 1],
                        accum_out=sg[:, col:col + 1])
                d = float(sch["damps"][r])
                K = float(sch["Ks"][r])
                nc.vector.tensor_scalar(cw[g], sg[g], 196.0, 0.5,
                                        op0=ALU.add, op1=ALU.mult)
                nc.vector.tensor_scalar(cw[g], cw[g], 15.0, 99.0,
                                        op0=ALU.max, op1=ALU.min)
                nc.vector.tensor_scalar(rw[g], cw[g], float(A5), float(A4),
                                        op0=ALU.mult, op1=ALU.add)
                nc.vector.tensor_tensor(rw2[g], rw[g], cw[g], op=ALU.mult)
                nc.vector.scalar_tensor_tensor(rw[g], rw2[g], float(A3), cw[g],
                                               op0=ALU.add, op1=ALU.mult)
                nc.vector.scalar_tensor_tensor(rw2[g], rw[g], float(A2), cw[g],
                                               op0=ALU.add, op1=ALU.mult)
                nc.vector.scalar_tensor_tensor(rw[g], rw2[g], float(A1), cw[g],
                                               op0=ALU.add, op1=ALU.mult)
                nc.vector.tensor_scalar(rw2[g], th[g], K, None, op0=ALU.add)
                nc.vector.scalar_tensor_tensor(th[g], rw[g], -d, rw2[g],
                                               op0=ALU.mult, op1=ALU.add)
                if r < 2:
                    nc.vector.tensor_scalar(thneg[g], th[g], -1.0, None,
                                            op0=ALU.mult)

            # ---- exact final count (DVE) ----
            for col in cols:
                nc.vector.tensor_scalar(junkD[:], slice_ap(col),
                                        th[:, col:col + 1], None,
                                        op0=ALU.is_ge, op1=ALU.add,
                                        accum_out=sg[:, col:col + 1])
            nc.vector.tensor_scalar(tm1[g], sg[g], -1.0, 48.0,
                                    op0=ALU.mult, op1=ALU.add)
            nc.vector.tensor_scalar(tm9[g], sg[g], -1.0, 40.0,
                                    op0=ALU.mult, op1=ALU.add)

            # ---- exact fixup: top-16 below theta + fused one-hot select ----
            for col in cols:
                tb = wk.tile([128, T], f32, name="tbw", tag="tbw")
                nc.vector.scalar_tensor_tensor(
                    tb[:], slice_ap(col), th[:, col:col + 1], slice_ap(col),
                    op0=ALU.is_lt, op1=ALU.mult)
                ma = mba[:, col * 8:(col + 1) * 8]
                mb_ = mbb[:, col * 8:(col + 1) * 8]
                nc.vector.max(ma, tb[:])
                nc.vector.match_replace(tb[:], ma, tb[:], NEG)
                nc.vector.max(mb_, tb[:])
                nc.vector.scalar_tensor_tensor(
                    junk8[:], iota8[:], tm1[:, col:col + 1], ma,
                    op0=ALU.is_equal, op1=ALU.mult,
                    accum_out=tsa[:, col:col + 1])
                nc.vector.scalar_tensor_tensor(
                    junk8[:], iota8[:], tm9[:, col:col + 1], mb_,
                    op0=ALU.is_equal, op1=ALU.mult,
                    accum_out=thstar[:, col:col + 1])
            nc.vector.tensor_tensor(thstar[g], thstar[g], tsa[g], op=ALU.add)

            # ---- bounce B-row thetas through DRAM to per-pair layout ----
            u0, u1 = (0, 5) if st == 0 else (5, 9)
            bcol0 = _col_of_bf(u0)
            dst = thbB[:].rearrange("(u q) -> q u", q=128)[:, u0:u1]
            nc.sync.dma_start(dst, thstar[:, bcol0:bcol0 + (u1 - u0)])
            src = thbB[0:NBROWS].rearrange("(p i) -> i p", p=NPAIR)
            nc.sync.dma_start(thB[:, 8 * st:8 * st + 8],
                              src[:, 8 * st:8 * st + 8])

            # ---- attention for this stage's pairs ----
            for p in pairs:
                bi, hh = divmod(p, H)
                qT = qkT_sb[hh]
                kT = qkT_sb[8 + hh]
                c0 = bi * T
                kTs = kT[:, c0:c0 + T]
                pcol = _col_of_pair(p)

                jA = jsb.tile([128, 208], bf16, name="jA", tag="jA")
                jB = jsb.tile([128, 208], bf16, name="jB", tag="jB")
                at_t = {}
                for blk, (P0, PN, PR, adj_sb, th_ap, rs) in enumerate([
                        (0, TA, 128, adjA_sb, thstar[:, pcol:pcol + 1], rsA),
                        (TA, TB, 80, adjB_sb, thB[:, p:p + 1], rsB)]):
                    s_ps = ps_s.tile([PN, T], f32, name="sps", tag="s")
                    nc.tensor.matmul(s_ps[:], qT[:, c0 + P0:c0 + P0 + PN], kTs,
                                     start=True, stop=True)
                    e_sb = esb.tile([PN, T], f32, name="et", tag="e")
                    nc.scalar.activation(e_sb[:], s_ps[:], AF.Exp)
                    ep_sb = epsb.tile([PN, T], f32, name="ept", tag="ep")
                    nc.vector.scalar_tensor_tensor(
                        ep_sb[:], adj_sb[:, p * T:(p + 1) * T], th_ap,
                        e_sb[:], op0=ALU.is_ge, op1=ALU.mult,
                        accum_out=rs[:, p:p + 1])
                    at = atsb.tile([PR, 256], bf16, name="att", tag=f"at{blk}")
                    nc.gpsimd.memset(at[:, T:256], 0.0)
                    if PR > PN:
                        nc.gpsimd.memset(at[64:PR, 0:T], 0.0)
                    nc.gpsimd.normalize_recip(at[0:PN, 0:T], ep_sb[:],
                                              rs[:, p:p + 1])
                    at_t[blk] = at
                # attn^T via DMA xbar transpose: garbage columns/rows land in
                # unread SBUF padding (jA/jB cols 196:208, jB partitions 68:128)
                atA, atB = at_t[0], at_t[1]
                nc.sync.dma_start(jA[:, 0:128], atA[:, 0:128], transpose=True)
                nc.sync.dma_start(jA[:, 128:208], atB[0:80, 0:128], transpose=True)
                nc.sync.dma_start(jB[:, 0:128], atA[:, 128:256], transpose=True)
                nc.sync.dma_start(jB[:, 128:208], atB[0:80, 128:256], transpose=True)

                oT_ps = ps_o.tile([D, T], f32, name="oTps", tag="oT")
                nc.tensor.matmul(oT_ps[:], vA_sb[bi][:, hh * D:(hh + 1) * D],
                                 jA[:, 0:T], start=True, stop=False)
                nc.tensor.matmul(oT_ps[:], vB_sb[bi][0:TB, hh * D:(hh + 1) * D],
                                 jB[0:TB, 0:T], start=False, stop=True)
                ot = oT_sb[hh // 2]
                r0 = (hh % 2) * D
                nc.scalar.activation(ot[r0:r0 + D, c0:c0 + T], oT_ps[:], AF.Copy)

        # ---------------- batched gelu sweep + final projection ----------------
        for kc in range(4):
            nc.scalar.activation(gT_sb[kc][:], oT_sb[kc][:],
                                 AF.Gelu if gelu else AF.Copy)
        for bi in range(NB):
            for (P0, PN) in [(0, TA), (TA, TB)]:
                ps = ps_f.tile([PN, DIM], f32, name="finps", tag="fin")
                for kc in range(4):
                    c0 = bi * T + P0
                    nc.tensor.matmul(ps[:], gT_sb[kc][:, c0:c0 + PN], wo_sb[kc][:],
                                     start=(kc == 0), stop=(kc == 3))
                o_sb = jsb.tile([PN, DIM], f32, name="osb", tag="osb")
                nc.scalar.activation(o_sb[:], ps[:], AF.Copy)
                nc.sync.dma_start(out_d[bi * T + P0: bi * T + P0 + PN, :], o_sb[:])

    nc.compile()
    return nc


def _prep_inputs(x, adj, Wqkv, Wv):
    """Host-side layout prep. Returns per-core in_maps."""
    x = np.asarray(x, np.float32)
    adj = np.asarray(adj, np.float32)
    Wqkv = np.asarray(Wqkv, np.float32)
    Wv = np.asarray(Wv, np.float32)

    Wh = Wqkv.reshape(DIM, H, 3 * D)
    wq = np.concatenate([Wh[:, hh, 0:D] for hh in range(H)], axis=1) * SCALE
    wk = np.concatenate([Wh[:, hh, D:2 * D] for hh in range(H)], axis=1)
    wv = np.concatenate([Wh[:, hh, 2 * D:3 * D] for hh in range(H)], axis=1)
    wqk = np.concatenate([wq, wk], axis=1)
    wqk_t = wqk.reshape(4, 128, 2 * DIM).astype(BF)
    wvp_t = wv.reshape(4, 128, DIM).astype(BF)
    wo_t = Wv.reshape(4, 128, DIM).astype(BF)
    iota8 = np.tile(np.arange(8, dtype=np.float32), (128, 1))

    in_maps = []
    for c in range(NCORES):
        xs = x[c * NB:(c + 1) * NB]
        xT = xs.transpose(2, 0, 1).reshape(DIM, NB * T)
        xT_t = xT.reshape(4, 128, NB * T).astype(BF)

        adj_c = adj[c * NB:(c + 1) * NB].reshape(NPAIR, T, T)
        adjA = np.ascontiguousarray(
            adj_c[:, 0:TA, :].transpose(1, 0, 2).reshape(TA, NPAIR * T))
        adjB = np.ascontiguousarray(
            adj_c[:, TA:T, :].transpose(1, 0, 2).reshape(TB, NPAIR * T))
        brows = adj_c[:, TA:T, :].reshape(NBROWS, T)
        bpad = np.zeros((NBF * 128, T), np.float32)
        bpad[:NBROWS] = brows
        adjBf = np.ascontiguousarray(
            bpad.reshape(NBF, 128, T).transpose(1, 0, 2).reshape(128, NBF * T))

        in_maps.append({
            "xT": xT_t, "wqk": wqk_t, "wvp": wvp_t, "wo": wo_t,
            "adjA": adjA, "adjB": adjB, "adjBf": adjBf, "iota8": iota8,
        })
    return in_maps


def kernel(x, adj, Wqkv, Wv, topk, _trace=False):
    assert int(topk) == TOPK
    in_maps = _prep_inputs(x, adj, Wqkv, Wv)
    if "nc" not in _PROGRAM_CACHE:
        _PROGRAM_CACHE["nc"] = _build_program()
    nc = _PROGRAM_CACHE["nc"]
    res = run_bass_kernel_spmd(nc, in_maps, core_ids=list(range(NCORES)),
                               trace=_trace)
    out = np.empty((B, T, DIM), np.float32)
    for c in range(NCORES):
        out[c * NB:(c + 1) * NB] = res.results[c]["out"].reshape(NB, T, DIM)
    kernel._last_results = res
    return out


# revision 16
# speedup vs baseline: 1.3108x; 1.3108x over previous
"""Trainium2 Bass kernel for nn_Attention_local (sparse routed attention).

Math (per batch b, head h):
  qkv = x @ Wqkv ; q,k,v per head (d=64)
  top-49 routing indices per (b,h,query) from adj logits
  attention over the selected 49 keys; gelu; @ Wv

Device strategy (8 cores, data-parallel over batch, 2 batches/core):
  - Replace the irregular gather with an exact threshold mask: per row,
    theta = 49th-largest of adj[b,h,i,:]; keep = adj >= theta selects
    exactly the top-49 set (validated: no ties at the boundary).
  - theta via 7x max8 + 6x match_replace rounds on DVE (fp32, exact).
  - s = (q*scale) @ k^T dense (bf16 matmul), e = exp(s) on ACT (|s|<0.5
    so no max-subtraction needed), ep = (adj>=theta)*e with fused row-sum
    on GPSIMD, normalize via gpsimd normalize_recip, transpose attn on PE,
    o^T = v^T-contract on PE, gelu on ACT, final projection on PE.
"""

import numpy as np
import ml_dtypes
from contextlib import ExitStack

import concourse.bass as bass
import concourse.tile as tile
from concourse import bacc, library_config, mybir
from concourse.bass_utils import run_bass_kernel_spmd

B, T, DIM = 16, 196, 512
H, D = 8, 64
TOPK = 49
NB = 2                 # batches per core
NPAIR = NB * H         # (b,h) pairs per core = 16
NCORES = 8
TA = 128               # query block A rows
TB = T - TA            # 68
NBF = 9                # flat selection tiles for B rows (16*68=1088 -> 9*128)
NBROWS = NPAIR * TB    # 1088
SCALE = DIM ** -0.5
BF = ml_dtypes.bfloat16
AF = mybir.ActivationFunctionType
ALU = mybir.AluOpType

# wave w handles pairs 4w..4w+3; B-flat tile bounds per wave (ceil(272(w+1)/128))
UB = [0, 3, 5, 7, 9]
NEG = -1.0e30

_PROGRAM_CACHE = {}


def _build_program(gelu=True):
    f32, bf16 = mybir.dt.float32, mybir.dt.bfloat16
    nc = bacc.Bacc("TRN2", target_bir_lowering=False, debug=False,
                   num_devices=NCORES)

    xT_d = nc.dram_tensor("xT", [4, 128, NB * T], bf16, kind="ExternalInput")
    wqk_d = nc.dram_tensor("wqk", [4, 128, 2 * DIM], bf16, kind="ExternalInput")
    wvp_d = nc.dram_tensor("wvp", [4, 128, DIM], bf16, kind="ExternalInput")
    wo_d = nc.dram_tensor("wo", [4, 128, DIM], bf16, kind="ExternalInput")
    adjA_d = nc.dram_tensor("adjA", [TA, NPAIR * T], f32, kind="ExternalInput")
    adjB_d = nc.dram_tensor("adjB", [TB, NPAIR * T], f32, kind="ExternalInput")
    adjBf_d = nc.dram_tensor("adjBf", [128, NBF * T], f32, kind="ExternalInput")
    id_d = nc.dram_tensor("ident", [128, 128], bf16, kind="ExternalInput")
    out_d = nc.dram_tensor("out", [NB * T, DIM], f32, kind="ExternalOutput")

    with ExitStack() as ctx:
        tc = ctx.enter_context(tile.TileContext(nc))
        const = ctx.enter_context(tc.tile_pool(name="const", bufs=1))
        dram = ctx.enter_context(tc.tile_pool(name="dram", bufs=1, space="DRAM"))
        mx = ctx.enter_context(tc.tile_pool(name="mx", bufs=4))
        esb = ctx.enter_context(tc.tile_pool(name="esb", bufs=8))
        epsb = ctx.enter_context(tc.tile_pool(name="epsb", bufs=4))
        atsb = ctx.enter_context(tc.tile_pool(name="atsb", bufs=4))
        jsb = ctx.enter_context(tc.tile_pool(name="jsb", bufs=3))
        ps_mm = ctx.enter_context(tc.tile_pool(name="ps_mm", bufs=1, space="PSUM"))
        ps_s = ctx.enter_context(tc.tile_pool(name="ps_s", bufs=3, space="PSUM"))
        ps_j = ctx.enter_context(tc.tile_pool(name="ps_j", bufs=2, space="PSUM"))
        ps_o = ctx.enter_context(tc.tile_pool(name="ps_o", bufs=1, space="PSUM"))
        ps_f = ctx.enter_context(tc.tile_pool(name="ps_f", bufs=1, space="PSUM"))

        nc.gpsimd.load_library(library_config.attn)

        # ---------------- small constant loads first (unblock PE) ----------
        xT_sb = [const.tile([128, NB * T], bf16, name=f"xT{kc}", tag=f"xT{kc}") for kc in range(4)]
        wqk_sb = [const.tile([128, 2 * DIM], bf16, name=f"wqk{kc}", tag=f"wqk{kc}") for kc in range(4)]
        wvp_sb = [const.tile([128, DIM], bf16, name=f"wvp{kc}", tag=f"wvp{kc}") for kc in range(4)]
        wo_sb = [const.tile([128, DIM], bf16, name=f"wo{kc}", tag=f"wo{kc}") for kc in range(4)]
        ident = const.tile([128, 128], bf16)
        nc.sync.dma_start(ident[:], id_d[:])
        for kc in range(4):
            nc.sync.dma_start(xT_sb[kc][:], xT_d[kc])
            nc.sync.dma_start(wqk_sb[kc][:], wqk_d[kc])
            nc.sync.dma_start(wvp_sb[kc][:], wvp_d[kc])
            nc.sync.dma_start(wo_sb[kc][:], wo_d[kc])

        adjA_sb = const.tile([TA, NPAIR * T], f32)      # mask compare, block A
        adjB_sb = const.tile([TB, NPAIR * T], f32)      # mask compare, block B
        adjA_wk = const.tile([TA, NPAIR * T], f32)      # selection scratch A
        adjB_wk = const.tile([128, NBF * T], f32)       # selection scratch B (flat)

        thA = const.tile([TA, NPAIR], f32)
        thB = const.tile([TB, NPAIR], f32)
        thBsel = const.tile([128, NBF], f32)
        thbB = dram.tile([NBF * 128], f32)
        rsA = const.tile([TA, NPAIR], f32)
        rsB = const.tile([TB, NPAIR], f32)

        # ---------------- q/k projection (PE): qT,kT head-major ----------------
        # wqk columns: [q of all heads (512) | k of all heads (512)], q pre-scaled.
        qkT_sb = [const.tile([D, NB * T], bf16, name=f"qkT{m}", tag=f"qkT{m}") for m in range(16)]
        # slots: 0..7 qT of head m ; 8..15 kT of head m-8
        for mt in range(8):
            ps = ps_mm.tile([128, NB * T], f32, name="qkps", tag="mm")
            for kc in range(4):
                nc.tensor.matmul(
                    ps[:], wqk_sb[kc][:, mt * 128:(mt + 1) * 128], xT_sb[kc][:],
                    start=(kc == 0), stop=(kc == 3))
            h0 = 2 * mt
            nc.scalar.activation(qkT_sb[h0][:], ps[0:D, :], AF.Copy)
            nc.scalar.activation(qkT_sb[h0 + 1][:], ps[D:128, :], AF.Copy)

        # ---------------- v projection (PE): v natural [token, DIM] ------------
        vA_sb = [const.tile([TA, DIM], bf16, name=f"vA{bi}", tag=f"vA{bi}") for bi in range(NB)]
        vB_sb = [const.tile([TB, DIM], bf16, name=f"vB{bi}", tag=f"vB{bi}") for bi in range(NB)]
        for bi in range(NB):
            psA = ps_mm.tile([TA, DIM], f32, name="vpsA", tag="mm")
            psB = ps_mm.tile([TB, DIM], f32, name="vpsB", tag="mm")
            for kc in range(4):
                c0 = bi * T
                nc.tensor.matmul(psA[:], xT_sb[kc][:, c0:c0 + TA], wvp_sb[kc][:],
                                 start=(kc == 0), stop=(kc == 3))
            for kc in range(4):
                c0 = bi * T + TA
                nc.tensor.matmul(psB[:], xT_sb[kc][:, c0:c0 + TB], wvp_sb[kc][:],
                                 start=(kc == 0), stop=(kc == 3))
            nc.scalar.activation(vA_sb[bi][:], psA[:], AF.Copy)
            nc.scalar.activation(vB_sb[bi][:], psB[:], AF.Copy)

        # ---------------- selection (DVE) ----------------
        def select49(seg, th_out):
            """th_out[:,0:1] <- 49th largest of each row of seg (destroys seg)."""
            m = mx.tile([seg.shape[0], 8], f32, name="m8", tag="m8")
            nc.vector.max(m[:], seg)
            for _ in range(6):
                nc.vector.match_replace(seg, m[:], seg, NEG)
                m = mx.tile([seg.shape[0], 8], f32, name="m8", tag="m8")
                nc.vector.max(m[:], seg)
            nc.vector.tensor_copy(th_out, m[:, 0:1])

        # oT staging (f32, same layout as gT) so gelu runs as one batched
        # sweep at the end -- avoids per-pair Exp<->Gelu ACT table reloads.
        oT_sb = [const.tile([128, NB * T], f32, name=f"oT{kc}", tag=f"oT{kc}") for kc in range(4)]
        gT_sb = [const.tile([128, NB * T], bf16, name=f"gT{kc}", tag=f"gT{kc}") for kc in range(4)]

        for w in range(4):
            pair_rng = range(4 * w, 4 * w + 4)
            # per-wave input DMAs (selection scratch first, then mask tiles)
            for p in pair_rng:
                nc.scalar.dma_start(adjA_wk[:, p * T:(p + 1) * T],
                                    adjA_d[:, p * T:(p + 1) * T])
            for u in range(UB[w], UB[w + 1]):
                nc.scalar.dma_start(adjB_wk[:, u * T:(u + 1) * T],
                                    adjBf_d[:, u * T:(u + 1) * T])
            for p in pair_rng:
                nc.scalar.dma_start(adjA_sb[:, p * T:(p + 1) * T],
                                    adjA_d[:, p * T:(p + 1) * T])
                nc.scalar.dma_start(adjB_sb[:, p * T:(p + 1) * T],
                                    adjB_d[:, p * T:(p + 1) * T])
            # selection
            for p in pair_rng:
                select49(adjA_wk[:, p * T:(p + 1) * T], thA[:, p:p + 1])
            for u in range(UB[w], UB[w + 1]):
                select49(adjB_wk[:, u * T:(u + 1) * T], thBsel[:, u:u + 1])
            # bounce theta for B rows through DRAM to re-pack per-pair
            u0, u1 = UB[w], UB[w + 1]
            dst = thbB[:].rearrange("(u q) -> q u", q=128)[:, u0:u1]
            nc.sync.dma_start(dst, thBsel[:, u0:u1])
            src = thbB[0:NBROWS].rearrange("(p i) -> i p", p=NPAIR)
            nc.sync.dma_start(thB[:, 4 * w:4 * w + 4], src[:, 4 * w:4 * w + 4])

            # attention for this wave
            for p in pair_rng:
                bi, hh = divmod(p, H)
                qT = qkT_sb[hh]
                kT = qkT_sb[8 + hh]
                c0 = bi * T
                kTs = kT[:, c0:c0 + T]

                j_ps = ps_j.tile([128, 2 * T], mybir.dt.bfloat16, name="jps", tag="j")
                jA_ps = j_ps[:, 0:T]
                jB_ps = j_ps[0:TB, T:2 * T]

                for blk, (P0, PN, adj_sb, th, rs) in enumerate(
                        [(0, TA, adjA_sb, thA, rsA), (TA, TB, adjB_sb, thB, rsB)]):
                    s_ps = ps_s.tile([PN, T], f32, name="sps", tag="s")
                    nc.tensor.matmul(s_ps[:], qT[:, c0 + P0:c0 + P0 + PN], kTs,
                                     start=True, stop=True)
                    e_sb = esb.tile([PN, T], f32, name="et", tag="e")
                    nc.scalar.activation(e_sb[:], s_ps[:], AF.Exp)
                    ep_sb = epsb.tile([PN, T], f32, name="ept", tag="ep")
                    nc.vector.scalar_tensor_tensor(
                        ep_sb[:], adj_sb[:, p * T:(p + 1) * T], th[:, p:p + 1],
                        e_sb[:], op0=ALU.is_ge, op1=ALU.mult,
                        accum_out=rs[:, p:p + 1])
                    at_sb = atsb.tile([PN, T], mybir.dt.bfloat16, name="att", tag="at")
                    nc.gpsimd.normalize_recip(at_sb[:], ep_sb[:], rs[:, p:p + 1])
                    nc.tensor.transpose(
                        jA_ps[:, P0:P0 + PN], at_sb[:, 0:TA], ident[0:PN, 0:PN])
                    nc.tensor.transpose(
                        jB_ps[:, P0:P0 + PN], at_sb[:, TA:T], ident[0:PN, 0:PN])

                jA_sb = jsb.tile([TA, T], mybir.dt.bfloat16, name="jAsb", tag="jAs")
                jB_sb = jsb.tile([TB, T], mybir.dt.bfloat16, name="jBsb", tag="jBs")
                nc.scalar.activation(jA_sb[:], jA_ps[:], AF.Copy)
                nc.scalar.activation(jB_sb[:], jB_ps[:], AF.Copy)

                oT_ps = ps_o.tile([D, T], f32, name="oTps", tag="oT")
                nc.tensor.matmul(oT_ps[:], vA_sb[bi][:, hh * D:(hh + 1) * D],
                                 jA_sb[:], start=True, stop=False)
                nc.tensor.matmul(oT_ps[:], vB_sb[bi][:, hh * D:(hh + 1) * D],
                                 jB_sb[:], start=False, stop=True)
                ot = oT_sb[hh // 2]
                r0 = (hh % 2) * D
                nc.scalar.activation(ot[r0:r0 + D, c0:c0 + T], oT_ps[:], AF.Copy)

            # per-batch gelu + final projection as soon as a batch completes
            if w in (1, 3):
                bi = w // 2
                cb = bi * T
                for kc in range(4):
                    nc.scalar.activation(gT_sb[kc][:, cb:cb + T],
                                         oT_sb[kc][:, cb:cb + T],
                                         AF.Gelu if gelu else AF.Copy)
                for (P0, PN) in [(0, TA), (TA, TB)]:
                    ps = ps_f.tile([PN, DIM], f32, name="finps", tag="fin")
                    for kc in range(4):
                        nc.tensor.matmul(ps[:], gT_sb[kc][:, cb + P0:cb + P0 + PN],
                                         wo_sb[kc][:], start=(kc == 0), stop=(kc == 3))
                    o_sb = jsb.tile([PN, DIM], f32, name="osb", tag="osb")
                    nc.scalar.activation(o_sb[:], ps[:], AF.Copy)
                    nc.sync.dma_start(out_d[cb + P0: cb + P0 + PN, :], o_sb[:])


    nc.compile()
    return nc


def _prep_inputs(x, adj, Wqkv, Wv):
    """Host-side layout prep. Returns per-core in_maps."""
    x = np.asarray(x, np.float32)
    adj = np.asarray(adj, np.float32)
    Wqkv = np.asarray(Wqkv, np.float32)
    Wv = np.asarray(Wv, np.float32)

    # head-major re-pack of Wqkv columns: [q all heads | k all heads], v separate
    Wh = Wqkv.reshape(DIM, H, 3 * D)
    wq = np.concatenate([Wh[:, hh, 0:D] for hh in range(H)], axis=1) * SCALE
    wk = np.concatenate([Wh[:, hh, D:2 * D] for hh in range(H)], axis=1)
    wv = np.concatenate([Wh[:, hh, 2 * D:3 * D] for hh in range(H)], axis=1)
    wqk = np.concatenate([wq, wk], axis=1)                    # [512, 1024]
    wqk_t = wqk.reshape(4, 128, 2 * DIM).astype(BF)
    wvp_t = wv.reshape(4, 128, DIM).astype(BF)
    wo_t = Wv.reshape(4, 128, DIM).astype(BF)
    ident = np.eye(128, dtype=BF)

    in_maps = []
    for c in range(NCORES):
        xs = x[c * NB:(c + 1) * NB]                           # [2,196,512]
        xT = xs.transpose(2, 0, 1).reshape(DIM, NB * T)       # [512, 392]
        xT_t = xT.reshape(4, 128, NB * T).astype(BF)

        adj_c = adj[c * NB:(c + 1) * NB].reshape(NPAIR, T, T)  # pair-major
        adjA = np.ascontiguousarray(
            adj_c[:, 0:TA, :].transpose(1, 0, 2).reshape(TA, NPAIR * T))
        adjB = np.ascontiguousarray(
            adj_c[:, TA:T, :].transpose(1, 0, 2).reshape(TB, NPAIR * T))
        # flat-packed B rows for selection: row rb = p*68 + (i-128)
        brows = adj_c[:, TA:T, :].reshape(NBROWS, T)
        bpad = np.zeros((NBF * 128, T), np.float32)
        bpad[:NBROWS] = brows
        adjBf = np.ascontiguousarray(
            bpad.reshape(NBF, 128, T).transpose(1, 0, 2).reshape(128, NBF * T))

        in_maps.append({
            "xT": xT_t, "wqk": wqk_t, "wvp": wvp_t, "wo": wo_t,
            "adjA": adjA, "adjB": adjB, "adjBf": adjBf, "ident": ident,
        })
    return in_maps


def kernel(x, adj, Wqkv, Wv, topk, _trace=False):
    assert int(topk) == TOPK
    in_maps = _prep_inputs(x, adj, Wqkv, Wv)
    if "nc" not in _PROGRAM_CACHE:
        _PROGRAM_CACHE["nc"] = _build_program()
    nc = _PROGRAM_CACHE["nc"]
    res = run_bass_kernel_spmd(nc, in_maps, core_ids=list(range(NCORES)),
                               trace=_trace)
    out = np.empty((B, T, DIM), np.float32)
    for c in range(NCORES):
        out[c * NB:(c + 1) * NB] = res.results[c]["out"].reshape(NB, T, DIM)
    kernel._last_results = res
    return out


# revision 17
# speedup vs baseline: 1.3195x; 1.0066x over previous
"""Trainium2 Bass kernel for nn_Attention_local (sparse routed attention).

Math (per batch b, head h):
  qkv = x @ Wqkv ; q,k,v per head (d=64)
  top-49 routing indices per (b,h,query) from adj logits
  attention over the selected 49 keys; gelu; @ Wv

Device strategy (8 cores, data-parallel over batch, 2 batches/core):
  - Replace the irregular gather with an exact threshold mask: per row,
    theta = 49th-largest of adj[b,h,i,:]; keep = adj >= theta selects
    exactly the top-49 set (validated: no ties at the boundary).
  - theta via 7x max8 + 6x match_replace rounds on DVE (fp32, exact).
  - s = (q*scale) @ k^T dense (bf16 matmul), e = exp(s) on ACT (|s|<0.5
    so no max-subtraction needed), ep = (adj>=theta)*e with fused row-sum
    on GPSIMD, normalize via gpsimd normalize_recip, transpose attn on PE,
    o^T = v^T-contract on PE, gelu on ACT, final projection on PE.
"""

import numpy as np
import ml_dtypes
from contextlib import ExitStack

import concourse.bass as bass
import concourse.tile as tile
from concourse import bacc, library_config, mybir
from concourse.bass_utils import run_bass_kernel_spmd

B, T, DIM = 16, 196, 512
H, D = 8, 64
TOPK = 49
NB = 2                 # batches per core
NPAIR = NB * H         # (b,h) pairs per core = 16
NCORES = 8
TA = 128               # query block A rows
TB = T - TA            # 68
NBF = 9                # flat selection tiles for B rows (16*68=1088 -> 9*128)
NBROWS = NPAIR * TB    # 1088
SCALE = DIM ** -0.5
BF = ml_dtypes.bfloat16
AF = mybir.ActivationFunctionType
ALU = mybir.AluOpType

# wave w handles pairs 4w..4w+3; B-flat tile bounds per wave (ceil(272(w+1)/128))
UB = [0, 3, 5, 7, 9]
NEG = -1.0e30

_PROGRAM_CACHE = {}


def _build_program(gelu=True):
    f32, bf16 = mybir.dt.float32, mybir.dt.bfloat16
    nc = bacc.Bacc("TRN2", target_bir_lowering=False, debug=False,
                   num_devices=NCORES)

    xT_d = nc.dram_tensor("xT", [4, 128, NB * T], bf16, kind="ExternalInput")
    wqk_d = nc.dram_tensor("wqk", [4, 128, 2 * DIM], bf16, kind="ExternalInput")
    wvp_d = nc.dram_tensor("wvp", [4, 128, DIM], bf16, kind="ExternalInput")
    wo_d = nc.dram_tensor("wo", [4, 128, DIM], bf16, kind="ExternalInput")
    adjA_d = nc.dram_tensor("adjA", [TA, NPAIR * T], f32, kind="ExternalInput")
    adjB_d = nc.dram_tensor("adjB", [TB, NPAIR * T], f32, kind="ExternalInput")
    adjBf_d = nc.dram_tensor("adjBf", [128, NBF * T], f32, kind="ExternalInput")
    id_d = nc.dram_tensor("ident", [128, 128], bf16, kind="ExternalInput")
    out_d = nc.dram_tensor("out", [NB * T, DIM], f32, kind="ExternalOutput")

    with ExitStack() as ctx:
        tc = ctx.enter_context(tile.TileContext(nc))
        const = ctx.enter_context(tc.tile_pool(name="const", bufs=1))
        dram = ctx.enter_context(tc.tile_pool(name="dram", bufs=1, space="DRAM"))
        mx = ctx.enter_context(tc.tile_pool(name="mx", bufs=4))
        rsp = ctx.enter_context(tc.tile_pool(name="rsp", bufs=8))
        esb = ctx.enter_context(tc.tile_pool(name="esb", bufs=8))
        epsb = ctx.enter_context(tc.tile_pool(name="epsb", bufs=4))
        atsb = ctx.enter_context(tc.tile_pool(name="atsb", bufs=4))
        jsb = ctx.enter_context(tc.tile_pool(name="jsb", bufs=3))
        ps_mm = ctx.enter_context(tc.tile_pool(name="ps_mm", bufs=1, space="PSUM"))
        ps_s = ctx.enter_context(tc.tile_pool(name="ps_s", bufs=3, space="PSUM"))
        ps_j = ctx.enter_context(tc.tile_pool(name="ps_j", bufs=2, space="PSUM"))
        ps_o = ctx.enter_context(tc.tile_pool(name="ps_o", bufs=1, space="PSUM"))
        ps_f = ctx.enter_context(tc.tile_pool(name="ps_f", bufs=1, space="PSUM"))

        nc.gpsimd.load_library(library_config.attn)

        # ---------------- small constant loads first (unblock PE) ----------
        xT_sb = [const.tile([128, NB * T], bf16, name=f"xT{kc}", tag=f"xT{kc}") for kc in range(4)]
        wqk_sb = [const.tile([128, 2 * DIM], bf16, name=f"wqk{kc}", tag=f"wqk{kc}") for kc in range(4)]
        wvp_sb = [const.tile([128, DIM], bf16, name=f"wvp{kc}", tag=f"wvp{kc}") for kc in range(4)]
        wo_sb = [const.tile([128, DIM], bf16, name=f"wo{kc}", tag=f"wo{kc}") for kc in range(4)]
        ident = const.tile([128, 128], bf16)
        nc.sync.dma_start(ident[:], id_d[:])
        for kc in range(4):
            nc.sync.dma_start(xT_sb[kc][:], xT_d[kc])
            nc.sync.dma_start(wqk_sb[kc][:], wqk_d[kc])
            nc.sync.dma_start(wvp_sb[kc][:], wvp_d[kc])
            nc.sync.dma_start(wo_sb[kc][:], wo_d[kc])

        adjA_sb = const.tile([TA, NPAIR * T], f32)      # mask compare, block A
        adjB_sb = const.tile([TB, NPAIR * T], f32)      # mask compare, block B
        adjA_wk = const.tile([TA, NPAIR * T], f32)      # selection scratch A
        adjB_wk = const.tile([128, NBF * T], f32)       # selection scratch B (flat)

        thA = const.tile([TA, NPAIR], f32)
        thB = const.tile([TB, NPAIR], f32)
        thBsel = const.tile([128, NBF], f32)
        thbB = dram.tile([NBF * 128], f32)

        # ---------------- q/k projection (PE): qT,kT head-major ----------------
        # wqk columns: [q of all heads (512) | k of all heads (512)], q pre-scaled.
        qkT_sb = [const.tile([D, NB * T], bf16, name=f"qkT{m}", tag=f"qkT{m}") for m in range(16)]
        # slots: 0..7 qT of head m ; 8..15 kT of head m-8
        for mt in range(8):
            ps = ps_mm.tile([128, NB * T], f32, name="qkps", tag="mm")
            for kc in range(4):
                nc.tensor.matmul(
                    ps[:], wqk_sb[kc][:, mt * 128:(mt + 1) * 128], xT_sb[kc][:],
                    start=(kc == 0), stop=(kc == 3))
            h0 = 2 * mt
            nc.scalar.activation(qkT_sb[h0][:], ps[0:D, :], AF.Copy)
            nc.scalar.activation(qkT_sb[h0 + 1][:], ps[D:128, :], AF.Copy)

        # ---------------- v projection (PE): v natural [token, DIM] ------------
        vA_sb = [const.tile([TA, DIM], bf16, name=f"vA{bi}", tag=f"vA{bi}") for bi in range(NB)]
        vB_sb = [const.tile([TB, DIM], bf16, name=f"vB{bi}", tag=f"vB{bi}") for bi in range(NB)]
        for bi in range(NB):
            psA = ps_mm.tile([TA, DIM], f32, name="vpsA", tag="mm")
            psB = ps_mm.tile([TB, DIM], f32, name="vpsB", tag="mm")
            for kc in range(4):
                c0 = bi * T
                nc.tensor.matmul(psA[:], xT_sb[kc][:, c0:c0 + TA], wvp_sb[kc][:],
                                 start=(kc == 0), stop=(kc == 3))
            for kc in range(4):
                c0 = bi * T + TA
                nc.tensor.matmul(psB[:], xT_sb[kc][:, c0:c0 + TB], wvp_sb[kc][:],
                                 start=(kc == 0), stop=(kc == 3))
            nc.scalar.activation(vA_sb[bi][:], psA[:], AF.Copy)
            nc.scalar.activation(vB_sb[bi][:], psB[:], AF.Copy)

        # ---------------- selection (DVE) ----------------
        def select49(seg, th_out):
            """th_out[:,0:1] <- 49th largest of each row of seg (destroys seg)."""
            m = mx.tile([seg.shape[0], 8], f32, name="m8", tag="m8")
            nc.vector.max(m[:], seg)
            for _ in range(6):
                nc.vector.match_replace(seg, m[:], seg, NEG)
                m = mx.tile([seg.shape[0], 8], f32, name="m8", tag="m8")
                nc.vector.max(m[:], seg)
            nc.vector.tensor_copy(th_out, m[:, 0:1])

        # oT staging (f32, same layout as gT) so gelu runs as one batched
        # sweep at the end -- avoids per-pair Exp<->Gelu ACT table reloads.
        oT_sb = [const.tile([128, NB * T], f32, name=f"oT{kc}", tag=f"oT{kc}") for kc in range(4)]
        gT_sb = [const.tile([128, NB * T], bf16, name=f"gT{kc}", tag=f"gT{kc}") for kc in range(4)]

        for w in range(4):
            pair_rng = range(4 * w, 4 * w + 4)
            # per-wave input DMAs (selection scratch first, then mask tiles)
            for p in pair_rng:
                nc.scalar.dma_start(adjA_wk[:, p * T:(p + 1) * T],
                                    adjA_d[:, p * T:(p + 1) * T])
            for u in range(UB[w], UB[w + 1]):
                nc.scalar.dma_start(adjB_wk[:, u * T:(u + 1) * T],
                                    adjBf_d[:, u * T:(u + 1) * T])
            for p in pair_rng:
                nc.scalar.dma_start(adjA_sb[:, p * T:(p + 1) * T],
                                    adjA_d[:, p * T:(p + 1) * T])
                nc.scalar.dma_start(adjB_sb[:, p * T:(p + 1) * T],
                                    adjB_d[:, p * T:(p + 1) * T])
            # selection
            for p in pair_rng:
                select49(adjA_wk[:, p * T:(p + 1) * T], thA[:, p:p + 1])
            for u in range(UB[w], UB[w + 1]):
                select49(adjB_wk[:, u * T:(u + 1) * T], thBsel[:, u:u + 1])
            # bounce theta for B rows through DRAM to re-pack per-pair
            u0, u1 = UB[w], UB[w + 1]
            dst = thbB[:].rearrange("(u q) -> q u", q=128)[:, u0:u1]
            nc.sync.dma_start(dst, thBsel[:, u0:u1])
            src = thbB[0:NBROWS].rearrange("(p i) -> i p", p=NPAIR)
            nc.sync.dma_start(thB[:, 4 * w:4 * w + 4], src[:, 4 * w:4 * w + 4])

            # attention for this wave
            for p in pair_rng:
                bi, hh = divmod(p, H)
                qT = qkT_sb[hh]
                kT = qkT_sb[8 + hh]
                c0 = bi * T
                kTs = kT[:, c0:c0 + T]

                j_ps = ps_j.tile([128, 2 * T], mybir.dt.bfloat16, name="jps", tag="j")
                jA_ps = j_ps[:, 0:T]
                jB_ps = j_ps[0:TB, T:2 * T]

                for blk, (P0, PN, adj_sb, th) in enumerate(
                        [(0, TA, adjA_sb, thA), (TA, TB, adjB_sb, thB)]):
                    s_ps = ps_s.tile([PN, T], f32, name="sps", tag="s")
                    nc.tensor.matmul(s_ps[:], qT[:, c0 + P0:c0 + P0 + PN], kTs,
                                     start=True, stop=True)
                    e_sb = esb.tile([PN, T], f32, name="et", tag="e")
                    nc.scalar.activation(e_sb[:], s_ps[:], AF.Exp)
                    ep_sb = epsb.tile([PN, T], f32, name="ept", tag="ep")
                    rs_t = rsp.tile([PN, 1], f32, name="rst", tag=f"rs{blk}")
                    nc.vector.scalar_tensor_tensor(
                        ep_sb[:], adj_sb[:, p * T:(p + 1) * T], th[:, p:p + 1],
                        e_sb[:], op0=ALU.is_ge, op1=ALU.mult,
                        accum_out=rs_t[:])
                    at_sb = atsb.tile([PN, T], mybir.dt.bfloat16, name="att", tag="at")
                    nc.gpsimd.normalize_recip(at_sb[:], ep_sb[:], rs_t[:])
                    nc.tensor.transpose(
                        jA_ps[:, P0:P0 + PN], at_sb[:, 0:TA], ident[0:PN, 0:PN])
                    nc.tensor.transpose(
                        jB_ps[:, P0:P0 + PN], at_sb[:, TA:T], ident[0:PN, 0:PN])

                jA_sb = jsb.tile([TA, T], mybir.dt.bfloat16, name="jAsb", tag="jAs")
                jB_sb = jsb.tile([TB, T], mybir.dt.bfloat16, name="jBsb", tag="jBs")
                nc.scalar.activation(jA_sb[:], jA_ps[:], AF.Copy)
                nc.scalar.activation(jB_sb[:], jB_ps[:], AF.Copy)

                oT_ps = ps_o.tile([D, T], f32, name="oTps", tag="oT")
                nc.tensor.matmul(oT_ps[:], vA_sb[bi][:, hh * D:(hh + 1) * D],
                                 jA_sb[:], start=True, stop=False)
                nc.tensor.matmul(oT_ps[:], vB_sb[bi][:, hh * D:(hh + 1) * D],
                                 jB_sb[:], start=False, stop=True)
                ot = oT_sb[hh // 2]
                r0 = (hh % 2) * D
                nc.scalar.activation(ot[r0:r0 + D, c0:c0 + T], oT_ps[:], AF.Copy)

            # per-batch gelu + final projection as soon as a batch completes
            if w in (1, 3):
                bi = w // 2
                cb = bi * T
                for kc in range(4):
                    nc.scalar.activation(gT_sb[kc][:, cb:cb + T],
                                         oT_sb[kc][:, cb:cb + T],
                                         AF.Gelu if gelu else AF.Copy)
                for (P0, PN) in [(0, TA), (TA, TB)]:
                    ps = ps_f.tile([PN, DIM], f32, name="finps", tag="fin")
                    for kc in range(4):
                        nc.tensor.matmul(ps[:], gT_sb[kc][:, cb + P0:cb + P0 + PN],
                                         wo_sb[kc][:], start=(kc == 0), stop=(kc == 3))
                    o_sb = jsb.tile([PN, DIM], f32, name="osb", tag="osb")
                    nc.scalar.activation(o_sb[:], ps[:], AF.Copy)
                    nc.sync.dma_start(out_d[cb + P0: cb + P0 + PN, :], o_sb[:])


    nc.compile()
    return nc


def _prep_inputs(x, adj, Wqkv, Wv):
    """Host-side layout prep. Returns per-core in_maps."""
    x = np.asarray(x, np.float32)
    adj = np.asarray(adj, np.float32)
    Wqkv = np.asarray(Wqkv, np.float32)
    Wv = np.asarray(Wv, np.float32)

    # head-major re-pack of Wqkv columns: [q all heads | k all heads], v separate
    Wh = Wqkv.reshape(DIM, H, 3 * D)
    wq = np.concatenate([Wh[:, hh, 0:D] for hh in range(H)], axis=1) * SCALE
    wk = np.concatenate([Wh[:, hh, D:2 * D] for hh in range(H)], axis=1)
    wv = np.concatenate([Wh[:, hh, 2 * D:3 * D] for hh in range(H)], axis=1)
    wqk = np.concatenate([wq, wk], axis=1)                    # [512, 1024]
    wqk_t = wqk.reshape(4, 128, 2 * DIM).astype(BF)
    wvp_t = wv.reshape(4, 128, DIM).astype(BF)
    wo_t = Wv.reshape(4, 128, DIM).astype(BF)
    ident = np.eye(128, dtype=BF)

    in_maps = []
    for c in range(NCORES):
        xs = x[c * NB:(c + 1) * NB]                           # [2,196,512]
        xT = xs.transpose(2, 0, 1).reshape(DIM, NB * T)       # [512, 392]
        xT_t = xT.reshape(4, 128, NB * T).astype(BF)

        adj_c = adj[c * NB:(c + 1) * NB].reshape(NPAIR, T, T)  # pair-major
        adjA = np.ascontiguousarray(
            adj_c[:, 0:TA, :].transpose(1, 0, 2).reshape(TA, NPAIR * T))
        adjB = np.ascontiguousarray(
            adj_c[:, TA:T, :].transpose(1, 0, 2).reshape(TB, NPAIR * T))
        # flat-packed B rows for selection: row rb = p*68 + (i-128)
        brows = adj_c[:, TA:T, :].reshape(NBROWS, T)
        bpad = np.zeros((NBF * 128, T), np.float32)
        bpad[:NBROWS] = brows
        adjBf = np.ascontiguousarray(
            bpad.reshape(NBF, 128, T).transpose(1, 0, 2).reshape(128, NBF * T))

        in_maps.append({
            "xT": xT_t, "wqk": wqk_t, "wvp": wvp_t, "wo": wo_t,
            "adjA": adjA, "adjB": adjB, "adjBf": adjBf, "ident": ident,
        })
    return in_maps


def kernel(x, adj, Wqkv, Wv, topk, _trace=False):
    assert int(topk) == TOPK
    in_maps = _prep_inputs(x, adj, Wqkv, Wv)
    if "nc" not in _PROGRAM_CACHE:
        _PROGRAM_CACHE["nc"] = _build_program()
    nc = _PROGRAM_CACHE["nc"]
    res = run_bass_kernel_spmd(nc, in_maps, core_ids=list(range(NCORES)),
                               trace=_trace)
    out = np.empty((B, T, DIM), np.float32)
    for c in range(NCORES):
        out[c * NB:(c + 1) * NB] = res.results[c]["out"].reshape(NB, T, DIM)
    kernel._last_results = res
    return out
